# revision 1
# baseline (speedup 1.0000x reference)
"""Trainium2 Bass kernel for LGAttention (global MHA + windowed local MHA).

Sharding: one attention head per NeuronCore (8 heads, 8 cores), SPMD.
Each core computes, for its head h:
  - global branch: q/k/v projections, flash-style softmax(q k^T)·v in
    "S^T layout" (k on partitions, q on free). The PV matmul uses v augmented
    with a ones column at col 64 (cols 48-63 zero-padded so the softmax
    denominator lands on partition 64, a legal base partition), producing
    unnormalized out^T plus the denominator in one pass.
  - local branch: same for the 128 independent 49-token windows.
  - output projection with the head's 48-row slice of proj_w (unnormalized).
Host: divides by the denominators, un-permutes the local branch, sums the
8 per-head partials, adds biases.
"""

import sys

sys.path.insert(0, "/opt/trn_rl_repo")

import numpy as np
import ml_dtypes

import concourse.bass as bass
import concourse.mybir as mybir
import concourse.tile as tile
from concourse import bacc, bass_utils

BF16 = mybir.dt.bfloat16
F32 = mybir.dt.float32

B, N, C = 2, 3136, 384
H, HD, WS = 8, 48, 7
NT = B * N            # 6272 tokens total
WT = WS * WS          # 49 tokens per window
QB = 448              # q-tile (free dim) for global attention
VS = 65               # v_aug column stride: 48 v + 16 pad + 1 ones
SCALE = float(HD) ** -0.5


def build_program():
    nc = bacc.Bacc(
        "TRN2",
        target_bir_lowering=False,
        debug=False,
        enable_asserts=False,
        num_devices=8,
    )

    din = {}
    for name, shape in [
        ("xT", (C, NT)), ("winT", (C, NT)),
        ("gwqk", (C, 112)), ("gwv", (C, HD)), ("gwp", (HD, C)),
        ("lwqk", (C, 112)), ("lwv", (C, HD)), ("lwp", (HD, C)),
    ]:
        din[name] = nc.dram_tensor(name, list(shape), BF16, kind="ExternalInput").ap()

    dout = {}
    for name, shape in [
        ("g_out", (NT, C)), ("l_out", (NT, C)),
        ("g_den", (1, NT)), ("l_den", (1, NT)),
    ]:
        dout[name] = nc.dram_tensor(name, list(shape), F32, kind="ExternalOutput").ap()

    with tile.TileContext(nc) as tc:
        _emit(tc, nc, din, dout)

    nc.compile()
    return nc


def _emit(tc, nc, din, dout):
    from contextlib import ExitStack

    ctx = ExitStack()
    with ctx:
        persist = ctx.enter_context(tc.tile_pool(name="persist", bufs=1))
        psum = ctx.enter_context(tc.tile_pool(name="psum", bufs=2, space="PSUM"))
        work = ctx.enter_context(tc.tile_pool(name="work", bufs=3))

        # ---- load inputs to SBUF ----
        xt = [persist.tile([128, NT], BF16, name=f"xt{c}") for c in range(3)]
        wt = [persist.tile([128, NT], BF16, name=f"wt{c}") for c in range(3)]
        for c in range(3):
            nc.sync.dma_start(xt[c][:, :], din["xT"][c * 128:(c + 1) * 128, :])
            nc.sync.dma_start(wt[c][:, :], din["winT"][c * 128:(c + 1) * 128, :])
        gwqk = persist.tile([128, 3 * 112], BF16, name="gwqk")
        lwqk = persist.tile([128, 3 * 112], BF16, name="lwqk")
        gwv = persist.tile([128, 3 * 48], BF16, name="gwv")
        lwv = persist.tile([128, 3 * 48], BF16, name="lwv")
        for c in range(3):
            nc.sync.dma_start(gwqk[:, c * 112:(c + 1) * 112], din["gwqk"][c * 128:(c + 1) * 128, :])
            nc.sync.dma_start(lwqk[:, c * 112:(c + 1) * 112], din["lwqk"][c * 128:(c + 1) * 128, :])
            nc.sync.dma_start(gwv[:, c * 48:(c + 1) * 48], din["gwv"][c * 128:(c + 1) * 128, :])
            nc.sync.dma_start(lwv[:, c * 48:(c + 1) * 48], din["lwv"][c * 128:(c + 1) * 128, :])
        gwp = persist.tile([HD, C], BF16, name="gwp")
        lwp = persist.tile([HD, C], BF16, name="lwp")
        nc.sync.dma_start(gwp[:, :], din["gwp"][:, :])
        nc.sync.dma_start(lwp[:, :], din["lwp"][:, :])

        # ---- persistent intermediates ----
        g_qT = persist.tile([HD, NT], BF16, name="g_qT")
        g_kT = persist.tile([HD, NT], BF16, name="g_kT")
        l_qT = persist.tile([HD, NT], BF16, name="l_qT")
        l_kT = persist.tile([HD, NT], BF16, name="l_kT")
        g_vaug = persist.tile([128, 50 * VS], BF16, name="g_vaug")  # 25 kb-blocks/batch
        l_vaug = persist.tile([49, 128 * VS], BF16, name="l_vaug")  # one block per window
        g_outT = persist.tile([HD, NT], BF16, name="g_outT")
        l_outT = persist.tile([HD, NT], BF16, name="l_outT")

        # v_aug pad/ones columns (softmax denominator comes out of the PV matmul)
        nc.vector.memset(g_vaug[:, :].rearrange("p (b k) -> p b k", k=VS)[:, :, 48:VS], 0.0)
        nc.vector.memset(l_vaug[:, :].rearrange("p (b k) -> p b k", k=VS)[:, :, 48:VS], 0.0)
        nc.vector.memset(g_vaug[:, :].rearrange("p (b k) -> p b k", k=VS)[:, :, 64:VS], 1.0)
        nc.vector.memset(l_vaug[:, :].rearrange("p (b k) -> p b k", k=VS)[:, :, 64:VS], 1.0)

        # ---- q/k projections: psum rows 0-47 = q, 64-111 = k (zero gap in W) ----
        for src, qT, kT, wqk in ((xt, g_qT, g_kT, gwqk), (wt, l_qT, l_kT, lwqk)):
            for qb in range(14):
                t0 = qb * QB
                ps = psum.tile([112, QB], F32, name="pqk", tag="pmix", bufs=4)
                for c in range(3):
                    nc.tensor.matmul(ps[:, :], wqk[:, c * 112:(c + 1) * 112],
                                     src[c][:, t0:t0 + QB], start=(c == 0), stop=(c == 2))
                nc.vector.tensor_copy(qT[:, t0:t0 + QB], ps[0:48, :])
                nc.vector.tensor_copy(kT[:, t0:t0 + QB], ps[64:112, :])

        # ---- v projections (token-major) ----
        for b in range(2):
            for j in range(25):
                sz = 128 if j < 24 else 64
                t0 = b * N + j * 128
                bl = b * 25 + j
                ps = psum.tile([128, HD], F32, name="pv", tag="pmix", bufs=4)
                for c in range(3):
                    nc.tensor.matmul(ps[0:sz, :], xt[c][:, t0:t0 + sz],
                                     gwv[:, c * 48:(c + 1) * 48], start=(c == 0), stop=(c == 2))
                nc.vector.tensor_copy(g_vaug[0:sz, bl * VS:bl * VS + 48], ps[0:sz, :])
        for w in range(64):
            t0 = w * 2 * WT
            ps = psum.tile([128, 2 * HD], F32, name="pvl", tag="pmix", bufs=4)
            for c in range(3):
                nc.tensor.matmul(ps[0:WT, 0:HD], wt[c][:, t0:t0 + WT],
                                 lwv[:, c * 48:(c + 1) * 48], start=(c == 0), stop=(c == 2))
            for c in range(3):
                nc.tensor.matmul(ps[0:WT, HD:2 * HD], wt[c][:, t0 + WT:t0 + 2 * WT],
                                 lwv[:, c * 48:(c + 1) * 48], start=(c == 0), stop=(c == 2))
            nc.vector.tensor_copy(l_vaug[0:WT, (2 * w) * VS:(2 * w) * VS + 48], ps[0:WT, 0:HD])
            nc.vector.tensor_copy(l_vaug[0:WT, (2 * w + 1) * VS:(2 * w + 1) * VS + 48], ps[0:WT, HD:2 * HD])

        # ---- local attention first: 16 groups of 8 windows ----
        for grp in range(16):
            psl = psum.tile([49, 392], F32, name="pSl", tag="pmix", bufs=4)
            for w8 in range(8):
                w = grp * 8 + w8
                t0 = w * WT
                nc.tensor.matmul(psl[:, w8 * WT:(w8 + 1) * WT],
                                 l_kT[:, t0:t0 + WT], l_qT[:, t0:t0 + WT],
                                 start=True, stop=True)
            exl = work.tile([49, 392], BF16, name="expSl")
            nc.scalar.activation(exl[:, :], psl[:, :],
                                 mybir.ActivationFunctionType.Exp, scale=SCALE)
            pol = psum.tile([VS, 392], F32, name="poutl", tag="pmix", bufs=4)
            for w8 in range(8):
                w = grp * 8 + w8
                nc.tensor.matmul(pol[:, w8 * WT:(w8 + 1) * WT],
                                 l_vaug[0:WT, w * VS:w * VS + VS],
                                 exl[:, w8 * WT:(w8 + 1) * WT], start=True, stop=True)
            nc.vector.tensor_copy(l_outT[:, grp * 392:(grp + 1) * 392], pol[0:48, :])
            dnl = work.tile([1, 392], F32, name="dnl", tag="dn", bufs=3)
            nc.vector.tensor_copy(dnl[:, :], pol[64:VS, :])
            nc.sync.dma_start(dout["l_den"][0:1, grp * 392:(grp + 1) * 392], dnl[:, :])

        # ---- global attention: qb pairs, one 896-wide exp per two S matmuls,
        # PV software-pipelined one kb iteration behind S so PE never stalls ----
        for b in range(2):
            for qp in range(4):
                qw = 448 if qp == 3 else 896
                nsub = qw // QB
                q0 = b * N + qp * 896
                po = [psum.tile([VS, QB], F32, name=f"po{s}", tag="pmix", bufs=4)
                      for s in range(nsub)]
                exs = [None] * 25
                for j in range(26):
                    if j < 25:
                        sz = 128 if j < 24 else 64
                        k0 = b * N + j * 128
                        ps = psum.tile([128, 1024], F32, name="pS", tag="pS", bufs=2)
                        for s in range(nsub):
                            nc.tensor.matmul(ps[0:sz, s * 512:s * 512 + QB],
                                             g_kT[:, k0:k0 + sz],
                                             g_qT[:, q0 + s * QB:q0 + (s + 1) * QB],
                                             start=True, stop=True)
                        ex = work.tile([128, 896], BF16, name="expS")
                        ps_v = ps[0:sz, :].rearrange("p (u k) -> p u k", k=512)[:, 0:nsub, 0:QB]
                        ex_v = ex[0:sz, 0:qw].rearrange("p (u k) -> p u k", k=QB)
                        nc.scalar.activation(ex_v, ps_v,
                                             mybir.ActivationFunctionType.Exp, scale=SCALE)
                        exs[j] = (ex, sz)
                    if j >= 1:
                        jj = j - 1
                        ex, sz = exs[jj]
                        bl = b * 25 + jj
                        for s in range(nsub):
                            nc.tensor.matmul(po[s][:, :],
                                             g_vaug[0:sz, bl * VS:bl * VS + VS],
                                             ex[0:sz, s * QB:(s + 1) * QB],
                                             start=(jj == 0), stop=(jj == 24))
                for s in range(nsub):
                    q0s = q0 + s * QB
                    nc.vector.tensor_copy(g_outT[:, q0s:q0s + QB], po[s][0:48, :])
                    dn = work.tile([1, QB], F32, name="dn", tag="dn", bufs=3)
                    nc.vector.tensor_copy(dn[:, :], po[s][64:VS, :])
                    nc.sync.dma_start(dout["g_den"][0:1, q0s:q0s + QB], dn[:, :])
                # interleave output projection for this region (l_outT is complete)
                for blk in range(qw // 112):
                    t0 = q0 + blk * 112
                    for outT, wp, dst in ((g_outT, gwp, dout["g_out"]),
                                          (l_outT, lwp, dout["l_out"])):
                        pp = psum.tile([112, C], F32, name="pp", tag="pmix", bufs=4)
                        nc.tensor.matmul(pp[:, :], outT[:, t0:t0 + 112], wp[:, :],
                                         start=True, stop=True)
                        sp = work.tile([112, C], F32, name="sproj", tag="sproj", bufs=4)
                        nc.vector.tensor_copy(sp[:, :], pp[:, :])
                        nc.sync.dma_start(dst[t0:t0 + 112, :], sp[:, :])


def _host_prep(x, g_qkv_w, g_proj_w, l_qkv_w, l_proj_w):
    bf = ml_dtypes.bfloat16
    xf = np.asarray(x, np.float32).reshape(NT, C)
    xT = np.ascontiguousarray(xf.T).astype(bf)
    x4 = np.asarray(x, np.float32).reshape(B, 56, 56, C)
    win = x4.reshape(B, 8, WS, 8, WS, C).transpose(0, 1, 3, 5, 2, 4)
    win = win.reshape(B, 8, 8, WS, WS, C).transpose(0, 1, 2, 4, 3, 5).reshape(NT, C)
    winT = np.ascontiguousarray(win.T).astype(bf)

    in_maps = []
    for h in range(8):
        m = {"xT": xT, "winT": winT}
        for pre, qkv_w, proj_w in (("g", g_qkv_w, g_proj_w), ("l", l_qkv_w, l_proj_w)):
            qw = np.asarray(qkv_w[:, h * HD:(h + 1) * HD], np.float32)
            kw = np.asarray(qkv_w[:, C + h * HD:C + (h + 1) * HD], np.float32)
            vw = np.asarray(qkv_w[:, 2 * C + h * HD:2 * C + (h + 1) * HD], np.float32)
            wqk = np.zeros((C, 112), np.float32)
            wqk[:, 0:48] = qw
            wqk[:, 64:112] = kw
            m[pre + "wqk"] = wqk.astype(bf)
            m[pre + "wv"] = np.ascontiguousarray(vw).astype(bf)
            m[pre + "wp"] = np.ascontiguousarray(
                np.asarray(proj_w, np.float32)[h * HD:(h + 1) * HD, :]).astype(bf)
        in_maps.append(m)
    return in_maps


_NC_CACHE = None


def kernel(x, g_qkv_w, g_proj_w, g_proj_b, l_qkv_w, l_proj_w, l_proj_b):
    global _NC_CACHE
    if _NC_CACHE is None:
        _NC_CACHE = build_program()
    nc = _NC_CACHE

    in_maps = _host_prep(x, g_qkv_w, g_proj_w, l_qkv_w, l_proj_w)
    res = bass_utils.run_bass_kernel_spmd(nc, in_maps, core_ids=list(range(8)))

    acc = np.zeros((NT, C), np.float32)
    l_acc = np.zeros((NT, C), np.float32)
    for h in range(8):
        r = res.results[h]
        acc += np.asarray(r["g_out"], np.float32) / np.asarray(r["g_den"], np.float32).reshape(NT, 1)
        l_acc += np.asarray(r["l_out"], np.float32) / np.asarray(r["l_den"], np.float32).reshape(NT, 1)
    l_tok = l_acc.reshape(B, 8, 8, WS, WS, C).transpose(0, 1, 3, 2, 4, 5).reshape(NT, C)
    out = acc + l_tok + np.asarray(g_proj_b, np.float32) + np.asarray(l_proj_b, np.float32)
    return out.reshape(B, N, C).astype(np.float32)



# revision 12
# speedup vs baseline: 1.1046x; 1.1046x over previous
"""Trainium2 Bass kernel for LGAttention (global MHA + windowed local MHA).

Sharding: one attention head per NeuronCore (8 heads, 8 cores), SPMD.
v2: 64x64 PE array tiling (4 concurrent matmul tiles) for the K=48 S-matmuls
and M=49 PV-matmuls; exp split between Scalar engine (exact) and Vector
engine (Schraudolph bf16 bit-trick) by kb-pair rotation; softmax denominator
rides as row 48 of the out^T tiles (bf16); out-projection evacuated by
direct PSUM->DRAM DMA. Host divides by denominators, un-permutes windows,
sums the 8 per-head partials, adds biases.
"""

import sys

sys.path.insert(0, "/opt/trn_rl_repo")

import numpy as np
import ml_dtypes

import concourse.bass as bass
import concourse.mybir as mybir
import concourse.tile as tile
from concourse import bacc, bass_utils

BF16 = mybir.dt.bfloat16
F32 = mybir.dt.float32
I16 = mybir.dt.int16

B, N, C = 2, 3136, 384
H, HD, WS = 8, 48, 7
NT = B * N            # 6272 tokens total
WT = WS * WS          # 49 tokens per window
QB = 448              # q-tile (free dim) for global attention
VS = 49               # v_aug column stride: 48 v + 1 ones (denominator row)
SCALE = float(HD) ** -0.5
# Schraudolph bf16 exp: bits = round(x*A + B); bitcast int16->bf16
SCH_A = 128.0 * 1.4426950408889634
SCH_B = 16250.5
EXP = mybir.ActivationFunctionType.Exp
MUL = mybir.AluOpType.mult
ADD = mybir.AluOpType.add
ACT_JP = (0, 3, 6, 9, 12)   # kb-pairs whose exp runs on the scalar engine


def build_program():
    nc = bacc.Bacc(
        "TRN2",
        target_bir_lowering=False,
        debug=False,
        enable_asserts=False,
        num_devices=8,
    )

    din = {}
    for name, shape in [
        ("xT", (C, NT)), ("winT", (C, NT)),
        ("gwqk", (C, 112)), ("gwv", (C, HD)), ("gwp", (112, C)),
        ("lwqk", (C, 112)), ("lwv", (C, HD)), ("lwp", (112, C)),
    ]:
        din[name] = nc.dram_tensor(name, list(shape), BF16, kind="ExternalInput").ap()

    dout = {}
    for name, shape, dt in [
        ("g_out", (NT, C), BF16), ("l_out", (NT, C), BF16),
        ("g_den", (2, NT), BF16), ("l_den", (2, NT), BF16),
    ]:
        dout[name] = nc.dram_tensor(name, list(shape), dt, kind="ExternalOutput").ap()

    with tile.TileContext(nc) as tc:
        _emit(tc, nc, din, dout)

    nc.compile()
    return nc


def _emit(tc, nc, din, dout):
    from contextlib import ExitStack

    ctx = ExitStack()
    with ctx:
        persist = ctx.enter_context(tc.tile_pool(name="persist", bufs=1))
        psum = ctx.enter_context(tc.tile_pool(name="psum", bufs=2, space="PSUM"))
        work = ctx.enter_context(tc.tile_pool(name="work", bufs=3))

        # ---- load inputs to SBUF ----
        xt = [persist.tile([128, NT], BF16, name=f"xt{c}") for c in range(3)]
        wt = [persist.tile([128, NT], BF16, name=f"wt{c}") for c in range(3)]
        for c in range(3):
            nc.sync.dma_start(xt[c][:, :], din["xT"][c * 128:(c + 1) * 128, :])
            nc.sync.dma_start(wt[c][:, :], din["winT"][c * 128:(c + 1) * 128, :])
        gwqk = persist.tile([128, 3 * 112], BF16, name="gwqk")
        lwqk = persist.tile([128, 3 * 112], BF16, name="lwqk")
        gwv = persist.tile([128, 3 * 48], BF16, name="gwv")
        lwv = persist.tile([128, 3 * 48], BF16, name="lwv")
        for c in range(3):
            nc.sync.dma_start(gwqk[:, c * 112:(c + 1) * 112], din["gwqk"][c * 128:(c + 1) * 128, :])
            nc.sync.dma_start(lwqk[:, c * 112:(c + 1) * 112], din["lwqk"][c * 128:(c + 1) * 128, :])
            nc.sync.dma_start(gwv[:, c * 48:(c + 1) * 48], din["gwv"][c * 128:(c + 1) * 128, :])
            nc.sync.dma_start(lwv[:, c * 48:(c + 1) * 48], din["lwv"][c * 128:(c + 1) * 128, :])
        gwp = persist.tile([112, C], BF16, name="gwp")
        lwp = persist.tile([112, C], BF16, name="lwp")
        nc.sync.dma_start(gwp[:, :], din["gwp"][:, :])
        nc.sync.dma_start(lwp[:, :], din["lwp"][:, :])

        # ---- persistent intermediates (q/k lo rows 0:48, hi rows 64:112) ----
        g_qT = persist.tile([128, NT], BF16, name="g_qT")
        g_kT = persist.tile([128, NT], BF16, name="g_kT")
        l_qT = persist.tile([128, NT], BF16, name="l_qT")
        l_kT = persist.tile([128, NT], BF16, name="l_kT")
        g_vaug = persist.tile([128, 50 * VS], BF16, name="g_vaug")
        l_vaug = persist.tile([128, 64 * VS], BF16, name="l_vaug")
        # out^T: lo half rows 0:48 (+den row 48), hi half rows 64:112 (+den 112);
        # the out-projection sums the halves via K=112 against [wp; 0; wp]
        g_outT = persist.tile([128, NT], BF16, name="g_outT")
        l_outT = persist.tile([128, NT], BF16, name="l_outT")
        nc.vector.memset(l_outT[:, :], 0.0)
        nc.vector.memset(g_outT[:, :], 0.0)

        # ones columns (softmax denominator comes out of the PV matmul, row 48)
        nc.vector.memset(g_vaug[:, :].rearrange("p (b k) -> p b k", k=VS)[:, :, 48:VS], 1.0)
        nc.vector.memset(l_vaug[:, :].rearrange("p (b k) -> p b k", k=VS)[:, :, 48:VS], 1.0)

        # ---- q/k projections: psum rows 0-47 = q, 64-111 = k ----
        for src, qT, kT, wqk in ((xt, g_qT, g_kT, gwqk), (wt, l_qT, l_kT, lwqk)):
            for qb in range(14):
                t0 = qb * QB
                ps = psum.tile([112, QB], F32, name="pqk", tag="pS", bufs=2)
                for c in range(3):
                    nc.tensor.matmul(ps[:, :], wqk[:, c * 112:(c + 1) * 112],
                                     src[c][:, t0:t0 + QB], start=(c == 0), stop=(c == 2))
                nc.scalar.copy(qT[0:48, t0:t0 + QB], ps[0:48, :])
                nc.vector.tensor_copy(kT[0:48, t0:t0 + QB], ps[64:112, :])
        # duplicate q/k into hi partitions (64:112) for the (64,*) PE tiles
        for t in (g_qT, g_kT, l_qT, l_kT):
            nc.sync.dma_start(t[64:112, :], t[0:48, :])

        # ---- global v projection (token-major), fills g_vaug cols 0:48 ----
        for b in range(2):
            for j in range(25):
                sz = 128 if j < 24 else 64
                t0 = b * N + j * 128
                bl = b * 25 + j
                ps = psum.tile([128, HD], F32, name="pv", tag="pS", bufs=2)
                for c in range(3):
                    nc.tensor.matmul(ps[0:sz, :], xt[c][:, t0:t0 + sz],
                                     gwv[:, c * 48:(c + 1) * 48], start=(c == 0), stop=(c == 2))
                nc.vector.tensor_copy(g_vaug[0:sz, bl * VS:bl * VS + 48], ps[0:sz, :])

        # ---- local v projection: col-tiled window pairs, 8 windows per psum ----
        for g8 in range(16):
            ps = psum.tile([128, 8 * 48], F32, name="pvl", tag="pS", bufs=2)
            for wi in range(8):
                w = g8 * 8 + wi
                t0 = w * WT
                r0 = 0 if w % 2 == 0 else 64
                for c in range(3):
                    nc.tensor.matmul(ps[r0:r0 + WT, wi * 48:(wi + 1) * 48],
                                     wt[c][:, t0:t0 + WT],
                                     lwv[:, c * 48:(c + 1) * 48], start=(c == 0), stop=(c == 2))
            dst_lo = l_vaug[0:WT, :].rearrange("p (w k) -> p w k", k=VS)[:, g8 * 4:(g8 + 1) * 4, 0:48]
            src_lo = ps[0:WT, :].rearrange("p (w k) -> p w k", k=48)[:, 0:8:2, :]
            dst_hi = l_vaug[64:64 + WT, :].rearrange("p (w k) -> p w k", k=VS)[:, g8 * 4:(g8 + 1) * 4, 0:48]
            src_hi = ps[64:64 + WT, :].rearrange("p (w k) -> p w k", k=48)[:, 1:8:2, :]
            nc.scalar.copy(dst_lo, src_lo)
            nc.vector.tensor_copy(dst_hi, src_hi)

        # ---- local attention: 8 iterations of 16 windows (2 S banks) ----
        for it in range(8):
            psA = psum.tile([128, 8 * WT], F32, name="pSlA", tag="pS", bufs=2)
            psB = psum.tile([128, 8 * WT], F32, name="pSlB", tag="pS", bufs=2)
            for wi in range(16):
                w = it * 16 + wi
                t0 = w * WT
                bank = psA if wi < 8 else psB
                r0, r1 = (0, 48) if wi < 8 else (64, 112)
                orow = 0 if wi % 2 == 0 else 64
                col = (wi % 8) * WT
                nc.tensor.matmul(bank[orow:orow + WT, col:col + WT],
                                 l_kT[r0:r1, t0:t0 + WT], l_qT[r0:r1, t0:t0 + WT],
                                 start=True, stop=True)
            exA = work.tile([128, 8 * WT], BF16, name="expSlA", tag="exl", bufs=3)
            exBi = work.tile([128, 8 * WT], I16, name="expSlB", tag="exl2", bufs=3)
            def _wv(t, r0, par):
                return t[r0:r0 + VS, :].rearrange("p (w k) -> p w k", k=WT)[:, par:8:2, :]
            nc.scalar.activation(_wv(exA, 0, 0), _wv(psA, 0, 0), EXP, scale=SCALE)
            nc.scalar.activation(_wv(exA, 64, 1), _wv(psA, 64, 1), EXP, scale=SCALE)
            nc.vector.tensor_scalar(_wv(exBi, 0, 0), _wv(psB, 0, 0),
                                    SCALE * SCH_A, SCH_B, MUL, ADD)
            nc.vector.tensor_scalar(_wv(exBi, 64, 1), _wv(psB, 64, 1),
                                    SCALE * SCH_A, SCH_B, MUL, ADD)
            poA = psum.tile([128, 8 * WT], F32, name="poutlA", tag="psO", bufs=4)
            poB = psum.tile([128, 8 * WT], F32, name="poutlB", tag="psO", bufs=4)
            for wi in range(16):
                w = it * 16 + wi
                po = poA if wi < 8 else poB
                col = (wi % 8) * WT
                vrow = 0 if w % 2 == 0 else 64
                rhs = (exA[vrow:vrow + WT, col:col + WT] if wi < 8 else
                       exBi[vrow:vrow + WT, col:col + WT].bitcast(BF16))
                nc.tensor.matmul(po[vrow:vrow + VS, col:col + WT],
                                 l_vaug[vrow:vrow + WT, (w // 2) * VS:(w // 2) * VS + VS],
                                 rhs, start=True, stop=True)
            # evacuate per parity half (even windows rows 0:49, odd rows 64:113)
            w0 = it * 16
            for po, base in ((poA, 0), (poB, 8)):
                wb = w0 + base
                for par, vrow, eng in ((0, 0, "s"), (1, 64, "v")):
                    src_o = po[vrow:vrow + VS, :].rearrange("p (w k) -> p w k", k=WT)[:, par:8:2, :]
                    dst_o = l_outT[vrow:vrow + VS, wb * WT:(wb + 8) * WT].rearrange(
                        "p (w k) -> p w k", k=WT)[:, par:8:2, :]
                    if eng == "s":
                        nc.scalar.copy(dst_o, src_o)
                    else:
                        nc.vector.tensor_copy(dst_o, src_o)
            nc.sync.dma_start(dout["l_den"][0:1, w0 * WT:(w0 + 16) * WT],
                              l_outT[48:49, w0 * WT:(w0 + 16) * WT])
            nc.sync.dma_start(dout["l_den"][1:2, w0 * WT:(w0 + 16) * WT],
                              l_outT[112:113, w0 * WT:(w0 + 16) * WT])

        # ---- global attention: kb pairs on 4 PE tiles; exp rotates between
        # scalar (exact) and vector (Schraudolph); PV pipelined one pair behind ----
        for b in range(2):
            for s in range(7):
                q0 = b * N + s * QB
                psOAlo = psum.tile([128, QB], F32, name="psOAlo", tag="psO", bufs=4)
                psOAhi = psum.tile([128, QB], F32, name="psOAhi", tag="psO", bufs=4)
                psOBhi = psum.tile([128, QB], F32, name="psOBhi", tag="psO", bufs=4)
                psOBlo = psum.tile([128, QB], F32, name="psOBlo", tag="psO", bufs=4)
                exs = [None] * 13
                for jp in range(14):
                    if jp < 13:
                        j0, j1 = 2 * jp, 2 * jp + 1
                        k0 = b * N + j0 * 128
                        k1 = b * N + j1 * 128
                        # two PSUM banks: S(j0) at cols 0:448 of bank 0,
                        # S(j1) at cols 512:960 of bank 1 (bank-aligned)
                        ps2 = psum.tile([128, 1024], F32, name="pS2", tag="pS", bufs=2)
                        nc.tensor.matmul(ps2[0:64, 0:QB], g_kT[0:48, k0:k0 + 64],
                                         g_qT[0:48, q0:q0 + QB], start=True, stop=True)
                        full0 = j0 < 24
                        if full0:
                            nc.tensor.matmul(ps2[64:128, 0:QB], g_kT[0:48, k0 + 64:k0 + 128],
                                             g_qT[0:48, q0:q0 + QB], start=True, stop=True)
                        have1 = j1 < 25
                        if have1:
                            nc.tensor.matmul(ps2[0:64, 512:512 + QB], g_kT[64:112, k1:k1 + 64],
                                             g_qT[64:112, q0:q0 + QB], start=True, stop=True)
                            nc.tensor.matmul(ps2[64:128, 512:512 + QB],
                                             g_kT[64:112, k1 + 64:k1 + 128],
                                             g_qT[64:112, q0:q0 + QB], start=True, stop=True)
                        rows = 128 if full0 else 64
                        nu = 2 if have1 else 1
                        ps_v = ps2[0:rows, :].rearrange("p (u k) -> p u k", k=512)[:, 0:nu, 0:QB]
                        if jp in ACT_JP:
                            ex = work.tile([128, 2 * QB], BF16, name="expA", tag="exA", bufs=3)
                            ex_v = ex[0:rows, :].rearrange("p (u k) -> p u k", k=QB)[:, 0:nu, :]
                            nc.scalar.activation(ex_v, ps_v, EXP, scale=SCALE)
                            exs[jp] = (ex, None)
                        else:
                            exi = work.tile([128, 2 * QB], I16, name="expB", tag="exB", bufs=3)
                            exi_v = exi[0:rows, :].rearrange("p (u k) -> p u k", k=QB)[:, 0:nu, :]
                            nc.vector.tensor_scalar(exi_v, ps_v,
                                                    SCALE * SCH_A, SCH_B, MUL, ADD)
                            exs[jp] = (None, exi)
                    if jp >= 1:
                        jj = jp - 1
                        exA, exi = exs[jj]
                        j0, j1 = 2 * jj, 2 * jj + 1
                        blA = b * 25 + j0
                        blB = b * 25 + j1

                        def ex_s(r0, r1, c0, c1):
                            if exA is not None:
                                return exA[r0:r1, c0:c1]
                            return exi[r0:r1, c0:c1].bitcast(BF16)

                        nc.tensor.matmul(psOAlo[0:VS, :], g_vaug[0:64, blA * VS:blA * VS + VS],
                                         ex_s(0, 64, 0, QB), start=(jj == 0), stop=(jj == 12))
                        if j0 < 24:
                            nc.tensor.matmul(psOAhi[64:64 + VS, :],
                                             g_vaug[64:128, blA * VS:blA * VS + VS],
                                             ex_s(64, 128, 0, QB),
                                             start=(jj == 0), stop=(jj == 11))
                        if j1 < 25:
                            nc.tensor.matmul(psOBhi[64:64 + VS, :],
                                             g_vaug[0:64, blB * VS:blB * VS + VS],
                                             ex_s(0, 64, QB, 2 * QB),
                                             start=(jj == 0), stop=(jj == 11))
                            nc.tensor.matmul(psOBlo[0:VS, :],
                                             g_vaug[64:128, blB * VS:blB * VS + VS],
                                             ex_s(64, 128, QB, 2 * QB),
                                             start=(jj == 0), stop=(jj == 11))
                # evacuate: psOA+psOB -> outT halves (dens land in rows 48/112);
                # two PSUM operands in one TensorTensor are illegal, so stage
                # psOB through SBUF on the scalar engine first
                t1 = work.tile([128, QB], F32, name="t1", tag="t1", bufs=2)
                nc.scalar.copy(t1[0:49, :], psOBlo[0:49, :])
                nc.scalar.copy(t1[64:113, :], psOBhi[64:113, :])
                nc.vector.tensor_tensor(g_outT[0:49, q0:q0 + QB],
                                        psOAlo[0:49, :], t1[0:49, :], ADD)
                nc.vector.tensor_tensor(g_outT[64:113, q0:q0 + QB],
                                        psOAhi[64:113, :], t1[64:113, :], ADD)
                nc.sync.dma_start(dout["g_den"][0:1, q0:q0 + QB],
                                  g_outT[48:49, q0:q0 + QB])
                nc.sync.dma_start(dout["g_den"][1:2, q0:q0 + QB],
                                  g_outT[112:113, q0:q0 + QB])
                # interleave output projection (direct PSUM->DRAM evacuation)
                for blk in range(4):
                    t0 = q0 + blk * 112
                    for outT, wp, dst, eng in ((g_outT, gwp, dout["g_out"], "v"),
                                               (l_outT, lwp, dout["l_out"], "s")):
                        pp = psum.tile([112, C], F32, name="pp", tag="pS", bufs=2)
                        nc.tensor.matmul(pp[:, :], outT[0:112, t0:t0 + 112], wp[:, :],
                                         start=True, stop=True)
                        sp = work.tile([112, C], BF16, name="sproj", tag="sproj", bufs=4)
                        if eng == "v":
                            nc.vector.tensor_copy(sp[:, :], pp[:, :])
                        else:
                            nc.scalar.copy(sp[:, :], pp[:, :])
                        nc.sync.dma_start(dst[t0:t0 + 112, :], sp[:, :])


def _host_prep(x, g_qkv_w, g_proj_w, l_qkv_w, l_proj_w):
    bf = ml_dtypes.bfloat16
    xf = np.asarray(x, np.float32).reshape(NT, C)
    xT = np.ascontiguousarray(xf.T).astype(bf)
    x4 = np.asarray(x, np.float32).reshape(B, 56, 56, C)
    win = x4.reshape(B, 8, WS, 8, WS, C).transpose(0, 1, 3, 5, 2, 4)
    win = win.reshape(B, 8, 8, WS, WS, C).transpose(0, 1, 2, 4, 3, 5).reshape(NT, C)
    winT = np.ascontiguousarray(win.T).astype(bf)

    in_maps = []
    for h in range(8):
        m = {"xT": xT, "winT": winT}
        for pre, qkv_w, proj_w in (("g", g_qkv_w, g_proj_w), ("l", l_qkv_w, l_proj_w)):
            qw = np.asarray(qkv_w[:, h * HD:(h + 1) * HD], np.float32)
            kw = np.asarray(qkv_w[:, C + h * HD:C + (h + 1) * HD], np.float32)
            vw = np.asarray(qkv_w[:, 2 * C + h * HD:2 * C + (h + 1) * HD], np.float32)
            wqk = np.zeros((C, 112), np.float32)
            wqk[:, 0:48] = qw
            wqk[:, 64:112] = kw
            m[pre + "wqk"] = wqk.astype(bf)
            m[pre + "wv"] = np.ascontiguousarray(vw).astype(bf)
            wph = np.asarray(proj_w, np.float32)[h * HD:(h + 1) * HD, :]
            wp2 = np.zeros((112, C), np.float32)
            wp2[0:48] = wph
            wp2[64:112] = wph
            m[pre + "wp"] = wp2.astype(bf)
        in_maps.append(m)
    return in_maps


_NC_CACHE = None


def kernel(x, g_qkv_w, g_proj_w, g_proj_b, l_qkv_w, l_proj_w, l_proj_b):
    global _NC_CACHE
    if _NC_CACHE is None:
        _NC_CACHE = build_program()
    nc = _NC_CACHE

    in_maps = _host_prep(x, g_qkv_w, g_proj_w, l_qkv_w, l_proj_w)
    res = bass_utils.run_bass_kernel_spmd(nc, in_maps, core_ids=list(range(8)))

    acc = np.zeros((NT, C), np.float32)
    l_acc = np.zeros((NT, C), np.float32)
    for h in range(8):
        r = res.results[h]
        gden = np.asarray(r["g_den"], np.float32).sum(0).reshape(NT, 1)
        lden = np.asarray(r["l_den"], np.float32).sum(0).reshape(NT, 1)
        acc += np.asarray(r["g_out"], np.float32) / gden
        l_acc += np.asarray(r["l_out"], np.float32) / lden
    l_tok = l_acc.reshape(B, 8, 8, WS, WS, C).transpose(0, 1, 3, 2, 4, 5).reshape(NT, C)
    out = acc + l_tok + np.asarray(g_proj_b, np.float32) + np.asarray(l_proj_b, np.float32)
    return out.reshape(B, N, C).astype(np.float32)
